# revision 8
# baseline (speedup 1.0000x reference)
"""Trainium2 Bass kernel for batched contrastive loss (InfoNCE over CxC sims).

Math (matches the jax reference):
    v_hat = v / ||v||,  t_hat = t / ||t||          (L2 over D, eps=1e-12)
    L[b,c,k] = (v_hat[b,c] . t_hat[b,k]) / 0.5     (logits)
    loss = mean_{b,c} [ logsumexp_k L[b,c,k] - L[b,c,c] ]

Strategy (8 NeuronCores, data-parallel over B=64 -> 8 batches/core), v3:
  - SWDGE DMA loads PAIRS of batches (f32 -> bf16 cast in the DMA datapath)
    into SBUF [128p=c%128, 2 batch, 4 c-chunk, 256 d].
  - Norms + positive logits via fused affine_mul_reduce (DVE): one op per
    (batch, chunk) computing (in0*scale+bias)*in1 with f32 row-sum
    accumulator.  Temperature 1/4 folds into the norm scale so
    sv = exp(-0.5*ln(0.25*|v|^2)) = 2/||v||; sv also rides the plog op's
    per-partition scale AP, so no separate V_hat tile is ever built.
  - rsqrt without Rsqrt and within ONE ACT table set
    (natural_log_exp_and_others, via the get_activation_tables patch
    below): one Ln + one Exp over [128,16] per pair.
  - T_hat = T*tsc via per-chunk tensor_scalar (4x DVE mode).
  - Transposes for the gram (d on partitions) via XBAR DMA transpose
    (SBUF->SBUF, sync HWDGE queue): one [128,1024] -> [128,8,128] transpose
    per (tensor, batch); no PE transposes, no PSUM->SBUF copies.  Block
    x=2j+e of the output is (chunk j, d-half e), so gram operands address
    it through a (j e) rearrange.
  - Grams: [128,512] f32 PSUM bank per chunk; exp on ACT with the sv scale
    AP; 3 of 4 chunks row-sum via the ACT accumulator, the 4th via DVE
    reduce to balance engine load.
  - ln(rowsums) and the subtraction hoisted out of the loop (one op each).
  - Each core returns per-(c,chunk,batch) loss terms [128,32]; host sums.
"""

import math
from contextlib import ExitStack

import numpy as np

import concourse.bacc as bacc
import concourse.bass as bass
import concourse.tile as tile
from concourse import mybir
from concourse.bass_utils import run_bass_kernel_spmd
from concourse.masks import make_identity

N_CORES = 8
B_PER_CORE = 8
PB = 2  # batches per DMA pair
NPAIR = B_PER_CORE // PB
C = 512
D = 256
P = 128
NCHUNK = C // P  # 4 c-chunks per batch
NDHALF = D // P  # 2 d-halves

F32 = mybir.dt.float32
BF16 = mybir.dt.bfloat16

USE_DMA_TRANSPOSE = True

# ---------------------------------------------------------------------------
# Keep ACT on a single table set: exp & ln both live in
# "natural_log_exp_and_others"; by removing them from every other set, the
# insert_act_table_loads fixpoint must pick that one set for both, so the
# kernel pays ONE table load instead of thrashing exp_and_others <->
# natural_log (~1.3us per reload).  Set indices are preserved (membership
# edited, nothing reordered).
_orig_get_tables = bacc.get_activation_tables


def _patched_get_tables(arch):
    tables = dict(_orig_get_tables(arch))
    keep = "natural_log_exp_and_others"
    strip = {mybir.ActivationFunctionType.Exp, mybir.ActivationFunctionType.Ln}
    if keep in tables:
        for name in tables:
            if name != keep:
                tables[name] = set(tables[name]) - strip
    return tables


bacc.get_activation_tables = _patched_get_tables


def _emit(ctx: ExitStack, tc: tile.TileContext, loss_ap, v_ap, t_ap):
    nc = tc.nc
    ctx.enter_context(nc.allow_low_precision("bf16 stream ops, f32 accums"))

    singles = ctx.enter_context(tc.tile_pool(name="singles", bufs=1))
    inputs = ctx.enter_context(tc.tile_pool(name="inputs", bufs=3))
    normed = ctx.enter_context(tc.tile_pool(name="normed", bufs=2))
    trans = ctx.enter_context(tc.tile_pool(name="trans", bufs=3))
    scratch = ctx.enter_context(tc.tile_pool(name="scratch", bufs=2))
    epool = ctx.enter_context(tc.tile_pool(name="epool", bufs=3))
    stats = ctx.enter_context(tc.tile_pool(name="stats", bufs=2))
    gp_pool = ctx.enter_context(tc.tile_pool(name="gp", bufs=6, space="PSUM"))
    if not USE_DMA_TRANSPOSE:
        tp_pool = ctx.enter_context(tc.tile_pool(name="tp", bufs=4, space="PSUM"))
        identity = singles.tile([P, P], BF16)
        make_identity(nc, identity)

    rs_all = singles.tile([P, NCHUNK * B_PER_CORE], F32)
    plog_all = singles.tile([P, NCHUNK * B_PER_CORE], F32)
    loss_cols = singles.tile([P, NCHUNK * B_PER_CORE], F32)

    for pair in range(NPAIR):
        b0 = pair * PB
        # ---- load a pair of batches (cast f32 -> bf16 in the DMA) ----
        V = inputs.tile([P, PB, NCHUNK, D], BF16, tag="V")
        T = inputs.tile([P, PB, NCHUNK, D], BF16, tag="T")
        nc.gpsimd.dma_start(
            out=V[:], in_=v_ap[b0 : b0 + PB].rearrange("b (n p) d -> p b n d", p=P)
        )
        nc.gpsimd.dma_start(
            out=T[:], in_=t_ap[b0 : b0 + PB].rearrange("b (n p) d -> p b n d", p=P)
        )

        # ---- transpose raw V for the gram (independent of norms) ----
        Vt = trans.tile([P, PB, 2 * NCHUNK, P], BF16, tag="Vt")
        if USE_DMA_TRANSPOSE:
            for pb in range(PB):
                nc.sync.dma_start_transpose(Vt[:, pb], V[:, pb])

        # ---- fused norms: nvt2[:, 0:8] = 0.25*|v|^2, [:, 8:16] = |t|^2 ----
        nvt2 = stats.tile([P, 2 * PB * NCHUNK], F32, tag="nvt2")
        sq = scratch.tile([P, PB, NCHUNK, D], BF16, tag="sq")
        sqt = scratch.tile([P, PB, NCHUNK, D], BF16, tag="sq")
        for pb in range(PB):
            for j in range(NCHUNK):
                g = pb * NCHUNK + j
                nc.vector.affine_mul_reduce(
                    out=sq[:, pb, j],
                    accum_out=nvt2[:, g : g + 1],
                    in0=V[:, pb, j],
                    in1=V[:, pb, j],
                    scale=0.25,
                    bias=0.0,
                )
                nc.vector.affine_mul_reduce(
                    out=sqt[:, pb, j],
                    accum_out=nvt2[:, PB * NCHUNK + g : PB * NCHUNK + g + 1],
                    in0=T[:, pb, j],
                    in1=T[:, pb, j],
                    scale=1.0,
                    bias=0.0,
                )

        # ---- scales on ACT (single table set):
        #   scl[:,0:8]  = exp(-0.5*ln(0.25*nv2)) = 2/||v||   (temp folded)
        #   scl[:,8:16] = exp(-0.5*ln(nt2))      = 1/||t||
        lnall = stats.tile([P, 2 * PB * NCHUNK], F32, tag="lnall")
        scl = stats.tile([P, 2 * PB * NCHUNK], F32, tag="scl")
        nc.scalar.activation(lnall[:], nvt2[:], mybir.ActivationFunctionType.Ln)
        nc.scalar.activation(
            scl[:], lnall[:], mybir.ActivationFunctionType.Exp, scale=-0.5
        )

        # ---- T_hat = T*tsc (per-chunk tensor_scalar, 4x DVE) ----
        Th = normed.tile([P, PB, NCHUNK, D], BF16, tag="Th")
        for pb in range(PB):
            for j in range(NCHUNK):
                g = pb * NCHUNK + j
                nc.vector.tensor_scalar_mul(
                    Th[:, pb, j],
                    T[:, pb, j],
                    scl[:, PB * NCHUNK + g : PB * NCHUNK + g + 1],
                )

        # ---- positive logits: plog = sum_d (V*sv)*Th, sv via scale AP ----
        sqp = scratch.tile([P, PB, NCHUNK, D], BF16, tag="sq")
        for pb in range(PB):
            for j in range(NCHUNK):
                g = pb * NCHUNK + j
                nc.vector.affine_mul_reduce(
                    out=sqp[:, pb, j],
                    accum_out=plog_all[:, (b0 + pb) * NCHUNK + j : (b0 + pb) * NCHUNK + j + 1],
                    in0=V[:, pb, j],
                    in1=Th[:, pb, j],
                    scale=scl[:, g : g + 1],
                    bias=0.0,
                )

        # ---- transpose T_hat ----
        Tt = trans.tile([P, PB, 2 * NCHUNK, P], BF16, tag="Tt")
        if USE_DMA_TRANSPOSE:
            for pb in range(PB):
                nc.sync.dma_start_transpose(Tt[:, pb], Th[:, pb])
        else:
            for pb in range(PB):
                tpv = tp_pool.tile([P, NDHALF, C], BF16, tag="tp")
                for e in range(NDHALF):
                    for j in range(NCHUNK):
                        nc.tensor.transpose(
                            tpv[:, e, j * P : (j + 1) * P],
                            V[:, pb, j, e * P : (e + 1) * P],
                            identity,
                        )
                nc.scalar.activation(
                    Vt[:, pb].rearrange("p (j e) c -> p e (j c)", e=NDHALF),
                    tpv[:],
                    mybir.ActivationFunctionType.Copy,
                )
                tpt = tp_pool.tile([P, NDHALF, C], BF16, tag="tp")
                for e in range(NDHALF):
                    for j in range(NCHUNK):
                        nc.tensor.transpose(
                            tpt[:, e, j * P : (j + 1) * P],
                            Th[:, pb, j, e * P : (e + 1) * P],
                            identity,
                        )
                nc.vector.tensor_copy(
                    out=Tt[:, pb].rearrange("p (j e) c -> p e (j c)", e=NDHALF),
                    in_=tpt[:],
                )

        # Block x = 2j+e of the DMA-transposed output is (chunk j, d-half e).
        Vtj = Vt.rearrange("p b (j e) c -> p b j e c", e=NDHALF)
        Tte = Tt.rearrange("p b (j e) c -> p b j e c", e=NDHALF)

        # ---- Grams (one PSUM bank per chunk) + exp; rowsums split 3:1 ----
        for pb in range(PB):
            b = b0 + pb
            for j in range(NCHUNK):
                gp = gp_pool.tile([P, C], F32, tag="gp")
                for e in range(NDHALF):
                    nc.tensor.matmul(
                        gp[:],
                        lhsT=Vtj[:, pb, j, e, :],
                        rhs=Tte[:, pb, :, e, :],
                        start=(e == 0),
                        stop=(e == NDHALF - 1),
                    )
                col = b * NCHUNK + j
                if j < NCHUNK - 1:
                    # exp + rowsum in one ACT op (fp32 accumulator)
                    E = epool.tile([P, C], BF16, tag="E")
                    nc.scalar.activation(
                        E[:],
                        gp[:],
                        mybir.ActivationFunctionType.Exp,
                        scale=scl[:, pb * NCHUNK + j : pb * NCHUNK + j + 1],
                        accum_out=rs_all[:, col : col + 1],
                    )
                else:
                    # exp on ACT, rowsum on DVE (engine balance)
                    E = epool.tile([P, C], BF16, tag="E")
                    nc.scalar.activation(
                        E[:],
                        gp[:],
                        mybir.ActivationFunctionType.Exp,
                        scale=scl[:, pb * NCHUNK + j : pb * NCHUNK + j + 1],
                    )
                    nc.vector.reduce_sum(
                        rs_all[:, col : col + 1], E[:], axis=mybir.AxisListType.X
                    )

    # ---- hoisted finals: loss = ln(rowsum) - plog, one op each ----
    lnr = singles.tile([P, NCHUNK * B_PER_CORE], F32)
    nc.scalar.activation(lnr[:], rs_all[:], mybir.ActivationFunctionType.Ln)
    nc.vector.tensor_sub(loss_cols[:], lnr[:], plog_all[:])
    nc.sync.dma_start(out=loss_ap, in_=loss_cols[:])


_NC_CACHE = []


def _get_nc():
    if not _NC_CACHE:
        nc = bacc.Bacc("TRN2", target_bir_lowering=False, debug=False)
        v_dram = nc.dram_tensor("v", [B_PER_CORE, C, D], F32, kind="ExternalInput")
        t_dram = nc.dram_tensor("t", [B_PER_CORE, C, D], F32, kind="ExternalInput")
        loss_dram = nc.dram_tensor(
            "loss", [P, NCHUNK * B_PER_CORE], F32, kind="ExternalOutput"
        )
        with tile.TileContext(nc) as tc, ExitStack() as ctx:
            _emit(ctx, tc, loss_dram.ap(), v_dram.ap(), t_dram.ap())
        nc.compile()
        _NC_CACHE.append(nc)
    return _NC_CACHE[0]


def kernel(visual_features, text_embeddings):
    v = np.ascontiguousarray(np.asarray(visual_features, dtype=np.float32))
    t = np.ascontiguousarray(np.asarray(text_embeddings, dtype=np.float32))
    v = v.reshape(N_CORES, B_PER_CORE, C, D)
    t = t.reshape(N_CORES, B_PER_CORE, C, D)
    in_maps = [{"v": v[i], "t": t[i]} for i in range(N_CORES)]
    nc = _get_nc()
    res = run_bass_kernel_spmd(nc, in_maps, list(range(N_CORES)))
    total = 0.0
    for r in res.results:
        total += float(r["loss"].astype(np.float64).sum())
    return np.float32(total / (N_CORES * B_PER_CORE * C))
